# revision 9
# baseline (speedup 1.0000x reference)
"""Single-head attention (B=8, S=2048, H=768, D=64) on 8 TRN2 NeuronCores.

Data-parallel over batch: core b computes batch element b end to end, no
collectives. Host pre-transposes inputs to [H, S] bf16 so every matmul
contraction is on the partition axis.

Schedule is built around three measured facts:
  * ACT is the softmax floor (32 exps of [128,1024]); it must start ASAP
    and never starve, and it runs ~20% faster when back-to-back;
  * the PE p-state ramps (0.65 -> 2.4 GHz over ~3us of continuous busy),
    so the PE stream must stay dense and never park behind a far-future
    DMA dependency (PE executes strictly in emission order);
  * aggregate input DMA is ~280 GB/s, so the 9.6 MB of input is ~34us of
    streaming - every DMA is emitted in consumption-deadline order.

Per-core dataflow (matmuls bf16 x bf16 -> f32 PSUM):
  qT[64,S], kT[64,S], vT[64,S] = W^T X    (6 h-tile accumulating matmuls
                                           per 512-col chunk, q/k/v split
                                           so each chunk unlocks on its
                                           own DMA)
  ps[128,1024] = kT_tile^T qT_half        (scores^T, sk on partitions)
  pth = exp(ps/8 + mask_bias)             (ACT, bf16, mask bias fused)
  vE[128, 65] tiles: PE-transpose of vT tiles; col 64 preset to ones
  po[c][65,512] += vE_t^T pth_t           (row 64 = softmax denominator)
  oT[65, S] f32 DMA'd out raw; host computes (oT[:64]/oT[64]).T
"""

import os
from contextlib import ExitStack

import numpy as np
import ml_dtypes

import concourse.bass as bass
import concourse.mybir as mybir
import concourse.tile as tile
from concourse import bacc
from concourse.bass_utils import run_bass_kernel_spmd
from concourse.masks import make_identity

S, H, D = 2048, 768, 64
P = 128
NT = S // P      # 16 sk tiles
HT = H // P      # 6 h tiles
CH = 512         # matmul moving-dim chunk
NCH = S // CH    # 4
HALF = 1024      # sq half width (exp tile width)
BF = mybir.dt.bfloat16
F32 = mybir.dt.float32
AF = mybir.ActivationFunctionType

LAST_RESULT = None  # BassKernelResults of the most recent run (for test.py)


def _build(debug=False):
    nc = bacc.Bacc()
    qT_d = nc.declare_dram_parameter("qT", [H, S], BF, isOutput=False)
    kT_d = nc.declare_dram_parameter("kT", [H, S], BF, isOutput=False)
    vT_d = nc.declare_dram_parameter("vT", [H, S], BF, isOutput=False)
    wqk_d = nc.declare_dram_parameter("wqk", [H, P], BF, isOutput=False)
    wv_d = nc.declare_dram_parameter("wv", [H, D], BF, isOutput=False)
    bq_d = nc.declare_dram_parameter("bq", [D, 1], F32, isOutput=False)
    bk_d = nc.declare_dram_parameter("bk", [D, 1], F32, isOutput=False)
    bv_d = nc.declare_dram_parameter("bv", [D, 1], F32, isOutput=False)
    mb_d = nc.declare_dram_parameter("mb", [P, NT], F32, isOutput=False)
    o_d = nc.declare_dram_parameter("o", [D + 1, S], F32, isOutput=True)

    with ExitStack() as ctx:
        tc = ctx.enter_context(tile.TileContext(nc))
        consts = ctx.enter_context(tc.tile_pool(name="consts", bufs=1))
        stage = ctx.enter_context(tc.tile_pool(name="stage", bufs=1))
        persist = ctx.enter_context(tc.tile_pool(name="persist", bufs=1))
        ppool = ctx.enter_context(tc.tile_pool(name="ppool", bufs=28))
        ostage = ctx.enter_context(tc.tile_pool(name="ostage", bufs=4))
        psA = ctx.enter_context(tc.tile_pool(name="psA", bufs=2, space="PSUM"))
        psO = ctx.enter_context(tc.tile_pool(name="psO", bufs=1, space="PSUM"))

        # ---- t=0: preload the ACT exp table with a dummy exp so the
        # ~1.4us table load happens during the DMA head, not before exp #1
        scr = consts.tile([P, 1], F32, tag="scr")
        nc.gpsimd.memset(scr, 0.0)
        dum = consts.tile([P, 1], BF, tag="dum")
        nc.scalar.activation(out=dum, in_=scr, func=AF.Exp, scale=1.0)

        # ---- consts + weights first on the sync ring
        mb_sb = consts.tile([P, NT], F32, tag="mb")
        nc.sync.dma_start(out=mb_sb, in_=mb_d[:, :])
        bq_sb = consts.tile([D, 1], F32, tag="bq")
        nc.sync.dma_start(out=bq_sb, in_=bq_d[:, :])
        bk_sb = consts.tile([D, 1], F32, tag="bk")
        nc.sync.dma_start(out=bk_sb, in_=bk_d[:, :])
        bv_sb = consts.tile([D, 1], F32, tag="bv")
        nc.sync.dma_start(out=bv_sb, in_=bv_d[:, :])
        w_sb = consts.tile([P, HT, P], BF, tag="w")  # [Wq|Wk] h-tiles
        nc.sync.dma_start(
            out=w_sb, in_=wqk_d[:, :].rearrange("(t p) n -> p t n", p=P)
        )
        wv_sb = consts.tile([P, HT, D], BF, tag="wv")
        nc.sync.dma_start(
            out=wv_sb, in_=wv_d[:, :].rearrange("(t p) n -> p t n", p=P)
        )
        ident_bf = consts.tile([P, P], BF, tag="ident_bf")
        make_identity(nc, ident_bf)

        # ---- input staging.  q/k as [128,512] chunk pieces so each proj
        # chunk unlocks on exactly its own bytes; v as [128,1024] halves.
        # All on the sync ring, emitted in consumption-deadline order:
        #   w, qk chunks 0-1 (first scores), k chunks 2-3 (kT tiles 8-15
        #   feed the half-0 scores), v half0, q chunk 2, q chunk 3 (rhs of
        #   half-1 scores), v half1.
        st = {}

        def stage_qk(t, h, c):
            tl = stage.tile(
                [P, CH], BF, tag="qk", bufs=48, name=f"st_{t}{h}{c}"
            )
            nc.sync.dma_start(
                out=tl,
                in_={"q": qT_d, "k": kT_d}[t][
                    h * P : (h + 1) * P, c * CH : (c + 1) * CH
                ],
            )
            st[t, h, c] = tl

        def stage_v(h, half):
            tl = stage.tile(
                [P, HALF], BF, tag="v", bufs=12, name=f"st_v{h}{half}"
            )
            nc.sync.dma_start(
                out=tl,
                in_=vT_d[h * P : (h + 1) * P, half * HALF : (half + 1) * HALF],
            )
            st["v", h, half] = tl

        for c in range(2):
            for h in range(HT):
                stage_qk("q", h, c)
            for h in range(HT):
                stage_qk("k", h, c)
        for c in range(2, 4):
            for h in range(HT):
                stage_qk("k", h, c)
        for h in range(HT):
            stage_v(h, 0)
        for c in range(2, 4):
            for h in range(HT):
                stage_qk("q", h, c)
        for h in range(HT):
            stage_v(h, 1)

        # ---- persistent SBUF ----
        qT_sb = persist.tile([D, S], BF, tag="qT")
        kT_sb = persist.tile([D, S], BF, tag="kT")
        vTp_sb = persist.tile([D, S], BF, tag="vTp")
        vE_sb = persist.tile([P, NT * (D + 1)], BF, tag="vE")
        nc.gpsimd.memset(vE_sb, 1.0)  # ones col (col 64 of each 65-tile)

        # ---- helper blocks -------------------------------------------
        def proj(t, c, tag):
            """One 512-col projection chunk of qT/kT/vT (6 accumulating
            matmuls + bias add). PSUM tag chosen so the o0-o3 bank
            rotation (projections -> po accumulators) follows program
            order."""
            w, wcols, bias, dst = {
                "q": (w_sb, slice(0, D), bq_sb, qT_sb),
                "k": (w_sb, slice(D, P), bk_sb, kT_sb),
                "v": (wv_sb, slice(0, D), bv_sb, vTp_sb),
            }[t]
            pp = psO.tile([D, CH], F32, tag=tag, name=f"pp_{t}{c}")
            for h in range(HT):
                if t == "v":
                    rhs = st["v", h, c // 2][:, (c % 2) * CH : (c % 2 + 1) * CH]
                else:
                    rhs = st[t, h, c]
                nc.tensor.matmul(
                    pp,
                    lhsT=w[:, h, wcols],
                    rhs=rhs,
                    start=(h == 0),
                    stop=(h == HT - 1),
                )
            nc.vector.tensor_scalar_add(
                out=dst[:, c * CH : (c + 1) * CH], in0=pp, scalar1=bias
            )

        pth = {}

        def scores_exp(t, half):
            ps = psA.tile([P, HALF], F32, tag="ps", name=f"ps{t}_{half}")
            for sub in range(2):
                nc.tensor.matmul(
                    ps[:, sub * CH : (sub + 1) * CH],
                    lhsT=kT_sb[:, t * P : (t + 1) * P],
                    rhs=qT_sb[
                        :, half * HALF + sub * CH : half * HALF + (sub + 1) * CH
                    ],
                    start=True,
                    stop=True,
                )
            pt = ppool.tile([P, HALF], BF, tag="pT", name=f"pt{t}_{half}")
            nc.scalar.activation(
                out=pt, in_=ps, func=AF.Exp, bias=mb_sb[:, t : t + 1], scale=0.125
            )
            pth[t, half] = pt

        def v_xpose(t, tag):
            """vT tile [64,128] -> vE tile [128,64] via PE transpose; the
            PSUM staging tile borrows an o-bank rotation slot right after
            the V-projection chunk that produced its input."""
            px = psO.tile([P, D], BF, tag=tag, name=f"px{t}")
            nc.tensor.transpose(
                px,
                in_=vTp_sb[:, t * P : (t + 1) * P],
                identity=ident_bf[:D, :D],
            )
            nc.vector.tensor_copy(
                out=vE_sb[:, t * (D + 1) : t * (D + 1) + D], in_=px
            )

        po = {}

        def mk_po(c):
            po[c] = psO.tile([D + 1, CH], F32, tag=f"o{c}", name=f"po{c}")

        def av(t, c):
            nc.tensor.matmul(
                po[c],
                lhsT=vE_sb[:, t * (D + 1) : (t + 1) * (D + 1)],
                rhs=pth[t, c // 2][:, (c % 2) * CH : (c % 2 + 1) * CH],
                start=(t == 0),
                stop=(t == NT - 1),
            )

        def emit_out(c):
            ot = ostage.tile([D + 1, CH], F32, tag="ot", name=f"ot{c}")
            nc.vector.tensor_copy(out=ot, in_=po[c])
            nc.sync.dma_start(out=o_d[:, c * CH : (c + 1) * CH], in_=ot)

        # ---- schedule -------------------------------------------------
        # PSUM o-bank rotation (each bufs=1):
        #   o0: Q0, K2, V0, px0-3,  po0     o1: K0, K3, V1, px4-7,  po1
        #   o2: Q1, Q2, V2, px8-11, po2     o3: K1, Q3, V3, px12-15, po3
        proj("q", 0, "o0")
        proj("k", 0, "o1")
        proj("q", 1, "o2")
        scores_exp(0, 0)
        proj("k", 1, "o3")
        scores_exp(1, 0)
        scores_exp(2, 0)
        scores_exp(3, 0)
        proj("k", 2, "o0")
        scores_exp(4, 0)
        scores_exp(5, 0)
        proj("k", 3, "o1")
        for t in range(6, 13):
            scores_exp(t, 0)
        proj("q", 2, "o2")
        scores_exp(13, 0)
        proj("v", 0, "o0")
        for t in range(0, 4):
            v_xpose(t, "o0")
        proj("q", 3, "o3")
        scores_exp(14, 0)
        proj("v", 1, "o1")
        for t in range(4, 8):
            v_xpose(t, "o1")
        scores_exp(15, 0)
        mk_po(0)
        mk_po(1)
        scores_exp(0, 1)
        scores_exp(1, 1)
        scores_exp(2, 1)
        av(0, 0)
        av(0, 1)
        scores_exp(3, 1)
        av(1, 0)
        av(1, 1)
        proj("v", 2, "o2")
        for t in range(8, 12):
            v_xpose(t, "o2")
        mk_po(2)
        scores_exp(4, 1)
        av(2, 0)
        av(2, 1)
        proj("v", 3, "o3")
        for t in range(12, 16):
            v_xpose(t, "o3")
        mk_po(3)
        scores_exp(5, 1)
        av(3, 0)
        av(3, 1)
        for j in range(6, NT):
            scores_exp(j, 1)
            av(j - 2, 0)
            av(j - 2, 1)
        for t in (NT - 2, NT - 1):
            av(t, 0)
            av(t, 1)
        emit_out(0)
        emit_out(1)
        # AV over half 1 (trails the half-1 exps).
        for t in range(NT):
            av(t, 2)
            av(t, 3)
        emit_out(2)
        emit_out(3)

        if debug:
            for nm, tl in [
                ("dbg_qT", qT_sb),
                ("dbg_kT", kT_sb),
                ("dbg_vTp", vTp_sb),
                ("dbg_vE", vE_sb),
            ]:
                dd = nc.declare_dram_parameter(
                    nm, list(tl.shape), BF, isOutput=True
                )
                nc.sync.dma_start(out=dd[:, :], in_=tl)

    return nc


_NC = None


def kernel(query, key, value, mask, Wq, bq, Wk, bk, Wv, bv):
    global _NC, LAST_RESULT
    bf16 = ml_dtypes.bfloat16
    B = query.shape[0]
    assert B == 8

    if _NC is None:
        _NC = _build(debug=bool(os.environ.get("KERNEL_DEBUG")))
        _NC.finalize()

    wqk = np.ascontiguousarray(
        np.concatenate([np.asarray(Wq), np.asarray(Wk)], axis=1).astype(bf16)
    )
    wv = np.ascontiguousarray(np.asarray(Wv).astype(bf16))
    bq_h = np.asarray(bq, np.float32).reshape(D, 1)
    bk_h = np.asarray(bk, np.float32).reshape(D, 1)
    bv_h = np.asarray(bv, np.float32).reshape(D, 1)

    in_maps = []
    for b in range(B):
        mb = ((np.asarray(mask[b], np.float32) - 1.0) * 1e9).reshape(NT, P).T
        in_maps.append(
            {
                "qT": np.ascontiguousarray(np.asarray(query[b]).T.astype(bf16)),
                "kT": np.ascontiguousarray(np.asarray(key[b]).T.astype(bf16)),
                "vT": np.ascontiguousarray(np.asarray(value[b]).T.astype(bf16)),
                "wqk": wqk,
                "wv": wv,
                "bq": bq_h,
                "bk": bk_h,
                "bv": bv_h,
                "mb": np.ascontiguousarray(mb),
            }
        )

    res = run_bass_kernel_spmd(
        _NC,
        in_maps,
        core_ids=list(range(8)),
        trace=bool(os.environ.get("KERNEL_TRACE")),
    )
    LAST_RESULT = res
    out = np.empty((B, S, D), np.float32)
    for b in range(B):
        oT = np.asarray(res.results[b]["o"])  # [65, S] f32, unnormalized
        out[b] = (oT[:D] / oT[D : D + 1]).T
    return out
